# revision 18
# baseline (speedup 1.0000x reference)
"""Trainium2 Bass kernel for nn_MaskedSelfAttention (causal, QK rms-norm).

Sharding: 8 cores = 2 (batch) x 4 (head groups of 4 heads).
Each core: qkv projection (bf16), causal attention with softmax denominator
collected via a ones-column in V, and a partial FC over its heads' feature
slice. Host sums the 4 bf16 partials per batch.

v2: bf16 datapath, software-pipelined attention j-loop, B+C interleaved
per 512-wide query block, normalization fused into the O^T psum eviction.
"""

import numpy as np
import ml_dtypes

import concourse.bacc as bacc
import concourse.mybir as mybir
import concourse.tile as tile
from concourse.bass_utils import run_bass_kernel_spmd

B, L, D = 2, 2048, 1024
DH = 64
NH = D // DH            # 16 heads total
P = 128
NHC = 4                 # heads per core
E3 = 3 * NHC * DH       # 768 qkv rows per core
LB = L // P             # 16 l-blocks
KB = D // P             # 8 contraction blocks
EPS = 1e-5
F32 = mybir.dt.float32
F32R = mybir.dt.float32r
BF16 = mybir.dt.bfloat16
FP8 = mybir.dt.float8e4
DR = mybir.MatmulPerfMode.DoubleRow

FX = mybir.ActivationFunctionType
MULT = mybir.AluOpType.mult

_CACHE = {}


def _build_nc():
    nc = bacc.Bacc("TRN2", target_bir_lowering=False, debug=False)

    xT = nc.dram_tensor("xT", (D, L), BF16, kind="ExternalInput").ap()
    wqkvT = nc.dram_tensor("wqkvT", (D, E3), BF16, kind="ExternalInput").ap()
    wfcT = nc.dram_tensor("wfcT", (NHC * DH, D), BF16, kind="ExternalInput").ap()
    triu = nc.dram_tensor("triu", (P, P), BF16, kind="ExternalInput").ap()
    wqk = nc.dram_tensor("wqk", (P, 1), F32, kind="ExternalInput").ap()
    ident = nc.dram_tensor("ident", (P, P), BF16, kind="ExternalInput").ap()
    outp = nc.dram_tensor("outp", (L, D), BF16, kind="ExternalOutput").ap()

    with tile.TileContext(nc) as tc:
        with (
            tc.tile_pool(name="cpool", bufs=1) as cpool,
            tc.tile_pool(name="wpool", bufs=1) as wpool,
            tc.tile_pool(name="ppool", bufs=1) as ppool,
            tc.tile_pool(name="xpool", bufs=4) as xpool,
            tc.tile_pool(name="work", bufs=8) as work,
            tc.tile_pool(name="ptpool", bufs=34) as ptpool,
            tc.tile_pool(name="opool", bufs=3) as opool,
        ):
            # ---- constants + weights (first x block + wqkv chunk 0 first) ----
            xT_r = xT.rearrange("(ko p) l -> p ko l", p=P)
            wqkvT_r = wqkvT.rearrange("(ko p) e -> p ko e", p=P)
            xc0 = xpool.tile([P, KB, P], BF16, tag="xc", name="xc_0")
            wqkv_sb = wpool.tile([P, KB, E3], BF16)
            nc.sync.dma_start(xc0[:, 0:1, :], xT_r[:, 0:1, 0:P])
            nc.sync.dma_start(wqkv_sb[:, 0:1, :], wqkvT_r[:, 0:1, :])
            nc.sync.dma_start(xc0[:, 1:KB, :], xT_r[:, 1:KB, 0:P])
            nc.sync.dma_start(wqkv_sb[:, 1:3, :], wqkvT_r[:, 1:3, :])
            nc.sync.dma_start(wqkv_sb[:, 3:KB, :], wqkvT_r[:, 3:KB, :])
            ident_sb = cpool.tile([P, P], BF16)
            nc.sync.dma_start(ident_sb, ident)
            triu_sb = cpool.tile([P, P], BF16)
            nc.sync.dma_start(triu_sb, triu)
            wqk_sb = cpool.tile([P, 1], F32)
            nc.sync.dma_start(wqk_sb, wqk)
            biasq = cpool.tile([P, 1], F32)
            nc.vector.memset(biasq, DH * EPS)

            # persistent activations, all bf16
            # qT/kT: [dh-of-2-heads, head-pair, l]
            qT = ppool.tile([P, 2, L], BF16)
            kT = ppool.tile([P, 2, L], BF16)
            vext = ppool.tile([P, LB, NHC, DH + 1], BF16)  # col DH = ones
            oT = ppool.tile([P, 2, L], BF16)               # normalized O^T

            onesf = cpool.tile([P, 1], F32)
            nc.vector.memset(onesf, 1.0)
            nc.vector.tensor_copy(
                vext[:, :, :, DH : DH + 1],
                onesf[:, :, None, None].to_broadcast((P, LB, NHC, 1)),
            )

            # ---- Phase A: qkv projection (l,e') + rms norm + transpose q,k ----
            with (
                tc.tile_pool(name="psA", bufs=2, space="PSUM") as psA,
                tc.tile_pool(name="psT", bufs=3, space="PSUM") as psT,
            ):
                pending = []

                def a_transposes(m, qn):
                    for g in range(4):  # blocks: 0,1 -> qT; 2,3 -> kT
                        tp = psT.tile([P, P], BF16, tag="tp", name=f"tp_{m}_{g}")
                        nc.tensor.transpose(tp, qn[:, g * P : (g + 1) * P], ident_sb)
                        if g < 2:
                            if g == 0:
                                nc.vector.tensor_copy(qT[:, g % 2, m * P : (m + 1) * P], tp)
                            else:
                                nc.scalar.copy(qT[:, g % 2, m * P : (m + 1) * P], tp)
                        else:
                            # fold q_norm_w * k_norm_w into kT (per-partition)
                            nc.vector.tensor_scalar_mul(
                                kT[:, g % 2, m * P : (m + 1) * P], tp, wqk_sb
                            )

                for m in range(LB):
                    if m == 0:
                        xc = xc0
                    else:
                        xc = xpool.tile([P, KB, P], BF16, tag="xc", name=f"xc_{m}")
                        nc.sync.dma_start(
                            xc, xT.rearrange("(ko p) l -> p ko l", p=P)[:, :, m * P : (m + 1) * P]
                        )
                    ps = psA.tile([P, 2 * NHC * DH], F32, tag="qkps", bufs=3, name=f"qkps_{m}")
                    psv = psA.tile([P, NHC * DH], F32, tag="vps", bufs=2, name=f"vps_{m}")
                    for k in range(KB):
                        nc.tensor.matmul(
                            ps,
                            lhsT=xc[:, k, :],
                            rhs=wqkv_sb[:, k, 0 : 2 * NHC * DH],
                            start=(k == 0),
                            stop=(k == KB - 1),
                        )
                    for k in range(KB):
                        nc.tensor.matmul(
                            psv,
                            lhsT=xc[:, k, :],
                            rhs=wqkv_sb[:, k, 2 * NHC * DH : 3 * NHC * DH],
                            start=(k == 0),
                            stop=(k == KB - 1),
                        )
                    nc.scalar.copy(
                        vext[:, m, :, 0:DH],
                        psv.rearrange("p (h d) -> p h d", d=DH),
                    )
                    sq = work.tile([P, 2 * NHC * DH], F32, tag="sq", name=f"sq_{m}")
                    nc.scalar.activation(sq, ps, FX.Square)
                    ssq = work.tile([P, 2 * NHC], F32, tag="ssq", name=f"ssq_{m}")
                    nc.vector.reduce_sum(
                        ssq,
                        sq.rearrange("p (h d) -> p h d", d=DH),
                        axis=mybir.AxisListType.X,
                    )
                    rin = work.tile([P, 2 * NHC], F32, tag="rin", name=f"rin_{m}")
                    # 1/rin = 0.125 / sqrt(mean + eps); the extra 1/64 vs the
                    # reference's 1/8 sdpa scale is undone by exp(scale=8)
                    nc.scalar.activation(rin, ssq, FX.Sqrt, bias=biasq[:, :], scale=1.0)
                    inv = work.tile([P, 2 * NHC], F32, tag="inv", name=f"inv_{m}")
                    nc.vector.reciprocal(inv, rin)
                    qn = work.tile([P, 2 * NHC * DH], BF16, tag="qn", name=f"qn_{m}")
                    nc.vector.tensor_tensor(
                        qn.rearrange("p (h d) -> p h d", d=DH),
                        ps.rearrange("p (h d) -> p h d", d=DH),
                        inv[:, :, None].to_broadcast((P, 2 * NHC, DH)),
                        MULT,
                    )
                    pending.append((m, qn))
                    if len(pending) > 1:
                        a_transposes(*pending.pop(0))
                a_transposes(*pending.pop(0))

                # fc weights: needed only in phase C; queue after the x blocks
                wfc_sb = wpool.tile([P, 2, D], BF16)
                nc.sync.dma_start(wfc_sb, wfcT.rearrange("(g p) e -> p g e", p=P))

            # ---- Phase B+C: attention + FC, software-pipelined across
            # (c, hp) units: S+exp of unit u+1 issue before the O-phase of
            # unit u, so the PE's O matmuls hide the next unit's exp latency.
            with (
                tc.tile_pool(name="psS", bufs=2, space="PSUM") as psS,
                tc.tile_pool(name="psO", bufs=2, space="PSUM") as psO,
            ):
                def sexp(c, hp):
                    nj = 4 * c + 4
                    sts = [None] * nj
                    pts = [None] * nj
                    def issue_S(j):
                        off = max(0, j * P - c * 512)
                        W = 512 - off
                        st = psS.tile([P, 2, 512], F32, tag="sT", name=f"sT_{hp}_{c}_{j}")
                        for h2 in range(2):
                            nc.tensor.matmul(
                                st[:, h2, 0:W],
                                lhsT=kT[h2 * DH : (h2 + 1) * DH, hp, j * P : (j + 1) * P],
                                rhs=qT[h2 * DH : (h2 + 1) * DH, hp, c * 512 + off : (c + 1) * 512],
                                start=True,
                                stop=True,
                            )
                        sts[j] = (st, off, W)
                    issue_S(0)
                    for j in range(nj):
                        st, off, W = sts[j]
                        pt = ptpool.tile([P, 2, 512], BF16, tag="pt", name=f"pt_{hp}_{c}_{j}")
                        nc.scalar.activation(pt[:, :, 0:W], st[:, :, 0:W], FX.Exp, scale=8.0)
                        if j >= 4 * c:
                            nc.vector.tensor_tensor(
                                pt[:, :, 0:P],
                                pt[:, :, 0:P],
                                triu_sb[:, None, :].to_broadcast((P, 2, P)),
                                MULT,
                            )
                        pts[j] = pt
                        if j + 1 < nj:
                            issue_S(j + 1)
                    return sts, pts

                def ophase(c, hp, sts, pts):
                    oAcc = [
                        psO.tile([P, 4, 72], F32, tag="oA", bufs=4, name=f"oA_{hp}_{c}_{h2}")
                        for h2 in range(2)
                    ]
                    for h2 in range(2):
                        for i in range(4):
                            for j in range(4 * c + i + 1):
                                off = sts[j][1]
                                nc.tensor.matmul(
                                    oAcc[h2][:, i, 0 : DH + 1],
                                    lhsT=pts[j][:, h2, i * P - off : (i + 1) * P - off],
                                    rhs=vext[:, j, 2 * hp + h2, :],
                                    start=(j == 0),
                                    stop=(j == 4 * c + i),
                                    skip_group_check=True,
                                )
                    return oAcc

                def tail(c, hp, oAcc):
                    # reciprocal of per-partition denominators, fused
                    # normalize on psum->sbuf eviction, transpose to oT
                    recc = work.tile([P, 2, 4], F32, tag="recc", bufs=2,
                                     name=f"recc_{hp}_{c}")
                    for h2 in range(2):
                        nc.vector.reciprocal(
                            recc[:, h2, :], oAcc[h2][:, :, DH : DH + 1].rearrange("p a b -> p (a b)")
                        )
                    osb = work.tile([P, 2, 4, DH], BF16, tag="osb", bufs=2,
                                    name=f"osb_{hp}_{c}")
                    for h2 in range(2):
                        nc.vector.tensor_tensor(
                            osb[:, h2, :, :],
                            oAcc[h2][:, :, 0:DH],
                            recc[:, h2, :, None].to_broadcast((P, 4, DH)),
                            MULT,
                        )
                    for mi in range(4):
                        m = 4 * c + mi
                        tpO = psS.tile([P, P], BF16, tag="sT", name=f"tpO_{hp}_{c}_{mi}")
                        for h2 in range(2):
                            nc.tensor.transpose(
                                tpO[h2 * DH : (h2 + 1) * DH, :],
                                osb[:, h2, mi, :],
                                ident_sb,
                            )
                        if mi % 2 == 0:
                            nc.scalar.copy(oT[:, hp, m * P : (m + 1) * P], tpO)
                        else:
                            nc.vector.tensor_copy(oT[:, hp, m * P : (m + 1) * P], tpO)
                        if hp == 1:
                            # FC for this l-block (both hp halves now done)
                            for n in range(2):
                                fp = psS.tile([P, 512], F32, tag="sT", name=f"fc_{m}_{n}")
                                for g in range(2):
                                    nc.tensor.matmul(
                                        fp,
                                        lhsT=oT[:, g, m * P : (m + 1) * P],
                                        rhs=wfc_sb[:, g, n * 512 : (n + 1) * 512],
                                        start=(g == 0),
                                        stop=(g == 1),
                                    )
                                ot = opool.tile([P, 512], BF16, tag="ot", name=f"ot_{m}_{n}")
                                if n == 0:
                                    nc.scalar.copy(ot, fp)
                                else:
                                    nc.vector.tensor_copy(ot, fp)
                                nc.sync.dma_start(outp[m * P : (m + 1) * P, n * 512 : (n + 1) * 512], ot)

                units = [(c, hp) for c in (3, 2, 1, 0) for hp in range(2)]
                prev = None
                for u in units:
                    cur = (u, sexp(*u))
                    if prev is not None:
                        (pc, php), (psts, ppts) = prev
                        oa = ophase(pc, php, psts, ppts)
                        tail(pc, php, oa)
                    prev = cur
                (pc, php), (psts, ppts) = prev
                oa = ophase(pc, php, psts, ppts)
                tail(pc, php, oa)

    nc.compile()
    return nc


def _make_in_maps(x, w_qkv, w_fc, q_norm_w, k_norm_w):
    bf16 = ml_dtypes.bfloat16
    fp8 = ml_dtypes.float8_e4m3fn
    triu_f = np.triu(np.ones((P, P), dtype=np.float32)).astype(bf16)
    ident = np.eye(P, dtype=np.float32).astype(bf16)

    wqk = np.tile((q_norm_w * k_norm_w).astype(np.float32), 2).reshape(P, 1)
    wqkvT = {}
    wfcTs = {}
    for hg in range(4):
        h0 = hg * NHC
        rows = np.concatenate(
            [
                w_qkv[h0 * DH : (h0 + NHC) * DH],
                w_qkv[D + h0 * DH : D + (h0 + NHC) * DH],
                w_qkv[2 * D + h0 * DH : 2 * D + (h0 + NHC) * DH],
            ],
            axis=0,
        )
        wqkvT[hg] = np.ascontiguousarray(rows.T).astype(bf16)
        wfcTs[hg] = np.ascontiguousarray(w_fc.T[h0 * DH : (h0 + NHC) * DH]).astype(bf16)
    xTs = [np.ascontiguousarray(x[b].T).astype(bf16) for b in range(B)]
    in_maps = []
    for core in range(8):
        b, hg = core // 4, core % 4
        in_maps.append(
            {
                "xT": xTs[b],
                "wqkvT": wqkvT[hg],
                "wfcT": wfcTs[hg],
                "triu": triu_f,
                "wqk": wqk,
                "ident": ident,
                
            }
        )
    return in_maps


def _is_causal(mask):
    idx = np.arange(mask.shape[0])
    return mask.shape == (L, L) and bool(np.all(mask == (idx[None, :] <= idx[:, None])))


def _reference_numpy(x, mask, w_qkv, w_fc, q_norm_w, k_norm_w, subset_attention_size):
    # slow but general fallback (only used if mask is not causal)
    b, l, d = x.shape
    qkv = x @ w_qkv.T
    q, k, v = np.split(qkv, 3, axis=-1)

    def heads(t):
        return t.reshape(b, l, NH, DH).transpose(0, 2, 1, 3)

    def rms(t, w):
        return t * (1.0 / np.sqrt(np.mean(t * t, -1, keepdims=True) + EPS)) * w

    q, k, v = heads(q), heads(k), heads(v)
    q, k = rms(q, q_norm_w), rms(k, k_norm_w)

    def sdpa(q, k, v, m):
        s = np.einsum("bhqd,bhkd->bhqk", q, k) / np.sqrt(DH)
        s = np.where(m[None, None], s, -1e30)
        s = s - s.max(-1, keepdims=True)
        p = np.exp(s)
        p /= p.sum(-1, keepdims=True)
        return np.einsum("bhqk,bhkd->bhqd", p, v)

    S = int(subset_attention_size) if subset_attention_size is not None else None
    if S is not None and S < l:
        o = np.concatenate(
            [
                sdpa(q[:, :, :S], k[:, :, :S], v[:, :, :S], mask[:S, :S]),
                sdpa(q[:, :, S:], k, v, mask[S:, :]),
            ],
            axis=2,
        )
    else:
        o = sdpa(q, k, v, mask)
    o = o.transpose(0, 2, 1, 3).reshape(b, l, d)
    return (o @ w_fc.T).astype(np.float32)


def kernel(**inputs):
    x = np.asarray(inputs["x"], dtype=np.float32)
    mask = np.asarray(inputs["mask"])
    w_qkv = np.asarray(inputs["w_qkv"], dtype=np.float32)
    w_fc = np.asarray(inputs["w_fc"], dtype=np.float32)
    q_norm_w = np.asarray(inputs["q_norm_w"], dtype=np.float32)
    k_norm_w = np.asarray(inputs["k_norm_w"], dtype=np.float32)

    if not _is_causal(mask):
        return _reference_numpy(
            x, mask, w_qkv, w_fc, q_norm_w, k_norm_w, inputs.get("subset_attention_size")
        )

    if "nc" not in _CACHE:
        _CACHE["nc"] = _build_nc()
    nc = _CACHE["nc"]

    in_maps = _make_in_maps(x, w_qkv, w_fc, q_norm_w, k_norm_w)
    res = run_bass_kernel_spmd(nc, in_maps, core_ids=list(range(8)))
    parts = [np.asarray(res.results[i]["outp"]).astype(np.float32) for i in range(8)]
    out = np.empty((B, L, D), dtype=np.float32)
    for b in range(B):
        out[b] = parts[b * 4] + parts[b * 4 + 1] + parts[b * 4 + 2] + parts[b * 4 + 3]
    return out
